# revision 27
# baseline (speedup 1.0000x reference)
"""Classical self-attention on 8 Trainium2 NeuronCores.

out = softmax((x Wq)(x Wk)^T / sqrt(D)) @ x   with x:[4,4096,1024] f32.

End-to-end wall time on this setup is dominated by the axon tunnel
(~38 MB/s), so the kernel is organized around minimizing host<->device
bytes; device compute (a few ms) is a rounding error by comparison.

Distribution: 8 shards = (batch, seq-half). Each core receives ONLY its
own 2048 rows of x, quantized to 20-bit fixed point over [-8,8) (uint16
plane with the top 16 bits + packed-nibble plane, 5 MB/core -> 40 MB
total instead of 128 MB f32-duplicated) plus a 1/8 row-slice of Wq/Wk
(1 MB/core, device-cached across calls by content hash). On device:
  - 8-way AllGather reassembles full Wq/Wk from the slices;
  - each core reconstructs its own x rows exactly (the 24-bit unpack is
    exact in f32), projects them to kT (fp16 hi/lo) and qT, and spills a
    fp16 copy of x (the V operand);
  - a pair AllGather (cores 2b, 2b+1) exchanges kT and the fp16 V copy
    over NeuronLink, giving every core all 4096 keys/values of its batch
    in rank order (softmax is permutation-invariant over keys, and the
    AV matmul uses the same gathered row order, so order drops out);
  - flash-style attention over the gathered keys, per 256-query block;
  - the output is quantized to uint8 (step 1/20, RNE) on device: 2 MB/core
    down instead of 8 MB. Dequantized on host; quant error ~0.025 abs vs a
    2e-2*absmax (~0.11) budget.

Precision: softmax logits have std ~1000, so the score path runs as fp16
hi/lo decompositions (a = hi + lo; a*b = ah*bh + ah*bl + al*bh, the al*bl
term dropped) carrying ~22 mantissa bits at full PE rate. The 20-bit x
quantization (step 2^-16) gives ~6e-3 rms logit error -> ~0.03 extra
absmax output error on near-tie softmax rows; with the output
quantization this totals ~9.5e-3 relative, 2x under the 2e-2 gate. The
unpack on device is exact in f32 (all power-of-two scales; the odd
nibble is recovered with the round-to-nearest f32->u8 conversion). The
AV matmul runs plain fp16.

Host path: the jit-wrapped bass_exec call is built ONCE and cached
(rebuilding it per call costs ~3s of XLA retrace/recompile), outputs are
custom-call results (no donated zero buffers to upload), packing and
uploads are pipelined per shard (device_put is async, so only the first
shard's packing is exposed), and the output fetch is pipelined with the
host-side dequantization. Per call: ~40 MB up + ~16 MB down at the
tunnel's ~35-50 MB/s, ~115 ms dispatch+execute (of which ~86 ms is the
axon round-trip floor).
"""

import hashlib
import types

import numpy as np

import concourse.bass as bass  # noqa: F401  (keeps the usual import env)
import concourse.mybir as mybir
import concourse.tile as tile
import concourse.bass2jax as bass2jax
from concourse import bacc
from concourse.masks import make_identity

# Problem constants (hardcoded: kernel.py must be self-contained).
B, S, D = 4, 4096, 1024
NCORES = 8
QH = S // 2            # rows (queries) per core
P = 128
NDC = D // P           # 8 d-chunks
SB = 256               # query superblock
NSB = QH // SB         # 8 superblocks per core
NKC = S // P           # 32 gathered key chunks
NOC = QH // P          # 16 own row chunks
JB = 512               # proj seq-block
NJ = QH // JB          # 4 own j-blocks
SCALE = 1.0 / float(np.sqrt(np.float32(D)))
HL = ((0, 0), (0, 1), (1, 0))  # hi/lo term pairs (lhs_split, rhs_split)

# Fixed point for x over [-8, 8). XBITS=24: u16 hi plane + u8 lo plane.
# XBITS=20: u16 plane (top 16 bits) + packed-nibble plane (2 els/byte).
XBITS = 20
# uint8 output quantization: u8 = round(out*OSCALE + 128). Range +-6.4
# comfortably covers |out| <= absmax(x) ~ 5.45 (outputs are convex
# combinations of x rows) while giving a 25% finer step than +-8.
OSCALE = 20.0
OBIAS = 128.0

F32 = mybir.dt.float32
F32R = mybir.dt.float32r
F16 = mybir.dt.float16
U16 = mybir.dt.uint16
U8 = mybir.dt.uint8
ALU = mybir.AluOpType
AX = mybir.AxisListType
AF = mybir.ActivationFunctionType

PAIR_GROUPS = [[0, 1], [2, 3], [4, 5], [6, 7]]
ALL_GROUP = [[0, 1, 2, 3, 4, 5, 6, 7]]


def _build_module():
    nc = bacc.Bacc(
        trn_type="TRN2",
        target_bir_lowering=False,
        debug=False,
        enable_asserts=False,
        num_devices=NCORES,
    )
    xh = nc.dram_tensor("xh", [QH, D], U16, kind="ExternalInput").ap()
    xl_cols = D if XBITS == 24 else D // 2
    xl = nc.dram_tensor("xl", [QH, xl_cols], U8, kind="ExternalInput").ap()
    wq = nc.dram_tensor("wq", [P, D], F32, kind="ExternalInput").ap()
    wk = nc.dram_tensor("wk", [P, D], F32, kind="ExternalInput").ap()
    out = nc.dram_tensor("out", [QH, D], U8, kind="ExternalOutput").ap()

    with tile.TileContext(nc) as tc:
        _emit(tc, nc, xh, xl, wq, wk, out)
    nc.compile()
    return nc


def _emit(tc, nc, xh, xl, wq, wk, out):
    ctx_pools = []

    def pool(**kw):
        p = tc.alloc_tile_pool(**kw)
        ctx_pools.append(p)
        return p

    # SBUF pools (per-partition KB in comments).
    stw_p = pool(name="stw", bufs=2)          # 2 x 32KB slots (W16 / ST shared)
    med_p = pool(name="med", bufs=2)          # 2 x 16KB (xt_j / qT)
    xs_p = pool(name="xs", bufs=3)            # 3 x 4KB (f32 chunk temps)
    xf_p = pool(name="xf", bufs=4)            # 4 x 2KB (fp16 staging/stream)
    xq_p = pool(name="xq", bufs=3)            # 3 x 2KB (u16/u8 plane loads)
    kf_p = pool(name="kf", bufs=3)            # 3 x 4KB (kT stream)
    out_p = pool(name="outp", bufs=2)         # 2 x 4KB (out f32 / spill staging)
    q8_p = pool(name="q8", bufs=2)            # 2 x 1KB (u8 out staging)
    msc_p = pool(name="msc", bufs=1)          # constants
    ms2_p = pool(name="ms2", bufs=2)          # rotating smalls

    # PSUM pools (8 banks total).
    p512 = pool(name="p512", bufs=2, space="PSUM")   # proj + AV [128,512]
    pst = pool(name="pst", bufs=2, space="PSUM")     # ST chunks [128,256]
    paux = pool(name="paux", bufs=2, space="PSUM")   # transposes / bcast
    psm = pool(name="psm", bufs=2, space="PSUM")     # row-sum accumulators

    # DRAM scratch.
    dram = pool(name="dram", bufs=1, space="DRAM")
    w_b = dram.tile([2, P, D], F32, tag="wb", name="w_b")
    w_all = dram.tile([NCORES, 2, P, D], F32, tag="wall", name="w_all")
    # kT (2*NDC*P = 2048 cols) and the fp16 V copy (D = 1024 cols) share one
    # tensor per key chunk so the pair exchange is a single AllGather.
    kv_own = dram.tile([NOC, P, 3 * D], F16, tag="kvo", name="kv_own")
    kv_all = dram.tile([NKC, P, 3 * D], F16, tag="kva", name="kv_all")
    qt_d = dram.tile([NSB, P, 2, NDC, SB], F16, tag="qtd", name="qt_d")

    def kt_own_ap(kc):
        return kv_own[kc, :, : 2 * D].rearrange(
            "p (hl dc k) -> p hl dc k", hl=2, dc=NDC
        )

    def kt_all_ap(kc):
        return kv_all[kc, :, : 2 * D].rearrange(
            "p (hl dc k) -> p hl dc k", hl=2, dc=NDC
        )

    # Constants.
    ident = msc_p.tile([P, P], F32, tag="ident", name="ident")
    make_identity(nc, ident)
    ident16 = msc_p.tile([P, P], F16, tag="ident16", name="ident16")
    nc.vector.tensor_copy(ident16, ident)
    negs32 = msc_p.tile([1, P], F32, tag="negs32", name="negs32")
    nc.gpsimd.memset(negs32, -SCALE)
    negscale = msc_p.tile([1, P], F32R, tag="negscale", name="negscale")
    nc.vector.tensor_copy(negscale, negs32)
    ones32 = msc_p.tile([P, 1], F32, tag="ones32", name="ones32")
    nc.gpsimd.memset(ones32, 1.0)
    ones16 = msc_p.tile([P, 1], F16, tag="ones16", name="ones16")
    nc.vector.tensor_copy(ones16, ones32)

    # ---------------- phase 0: weight gather + split ----------------
    nc.sync.dma_start(w_b[0], wq)
    nc.sync.dma_start(w_b[1], wk)
    nc.gpsimd.collective_compute(
        kind="AllGather", op=ALU.bypass, replica_groups=ALL_GROUP,
        ins=[w_b], outs=[w_all],
    )

    # Weights as fp16 hi/lo: w16[:, hl, din_chunk, dout].
    wq_t = stw_p.tile([P, 2, NDC, D], F16, tag="stw", name="wq_t")
    wk_t = stw_p.tile([P, 2, NDC, D], F16, tag="stw", name="wk_t")
    for w_dst, wi, wn in ((wq_t, 0, "q"), (wk_t, 1, "k")):
        for i in range(NDC):
            w_in = xs_p.tile([P, D], F32, tag="xs", name=f"w{wn}in{i}")
            nc.sync.dma_start(w_in, w_all[i, wi])
            nc.scalar.copy(w_dst[:, 0, i, :], w_in)
            nc.vector.tensor_tensor(
                w_dst[:, 1, i, :], w_in, w_dst[:, 0, i, :], ALU.subtract
            )

    # ---------------- phase 1: unpack + projections (own rows) ----------
    for j in range(NJ):
        xt_j = med_p.tile([P, 2, NDC, JB], F16, tag="med", name=f"xt{j}")
        for sc in range(JB // P):
            kc = j * (JB // P) + sc
            row0 = kc * P
            xh_t = xq_p.tile([P, D], U16, tag="xq", name=f"xh{kc}")
            nc.sync.dma_start(xh_t, xh[row0 : row0 + P, :])
            a_hi = xs_p.tile([P, D], F32, tag="xs", name=f"ah{kc}")
            x_f = xs_p.tile([P, D], F32, tag="xs", name=f"xr{kc}")
            if XBITS == 24:
                # exact unpack: x = uh*2^-12 - 8 + ul*2^-20
                xl_t = xq_p.tile([P, D], U8, tag="xq", name=f"xl{kc}")
                nc.sync.dma_start(xl_t, xl[row0 : row0 + P, :])
                a_lo = xs_p.tile([P, D], F32, tag="xs", name=f"al{kc}")
                nc.scalar.activation(
                    a_hi, xh_t, AF.Copy, bias=-8.0, scale=2.0 ** -12
                )
                nc.scalar.activation(
                    a_lo, xl_t, AF.Copy, bias=0.0, scale=2.0 ** -20
                )
                nc.vector.tensor_tensor(x_f, a_hi, a_lo, ALU.add)
            else:
                # x = A*2^-12 - 8 + n_even*2^-16 (even cols) / n_odd*2^-16
                # (odd cols), nibbles n packed two-per-byte in xl. The odd
                # nibble is recovered as h' = rne(n/16 + 1 - 0.46875) =
                # n_odd + 1 via the (round-to-nearest) f32->u8 conversion;
                # every step below is exact in f32.
                xl_t = xq_p.tile([P, D // 2], U8, tag="xq", name=f"xl{kc}")
                nc.sync.dma_start(xl_t, xl[row0 : row0 + P, :])
                nc.scalar.activation(
                    a_hi, xh_t, AF.Copy, bias=-8.0, scale=2.0 ** -12
                )
                n16 = xs_p.tile([P, D // 2], F32, tag="nib", name=f"n16_{kc}")
                hp = xq_p.tile([P, D // 2], U8, tag="hp", name=f"hp{kc}")
                h16 = xs_p.tile([P, D // 2], F32, tag="nib", name=f"h16_{kc}")
                h12 = xs_p.tile([P, D // 2], F32, tag="nib", name=f"h12_{kc}")
                l16 = xs_p.tile([P, D // 2], F32, tag="nib", name=f"l16_{kc}")
                nc.scalar.activation(
                    n16, xl_t, AF.Copy, bias=0.0, scale=2.0 ** -16
                )
                nc.scalar.activation(
                    hp, xl_t, AF.Copy, bias=1.0 - 0.46875, scale=2.0 ** -4
                )
                nc.scalar.activation(
                    h16, hp, AF.Copy, bias=-(2.0 ** -16), scale=2.0 ** -16
                )
                nc.scalar.activation(
                    h12, hp, AF.Copy, bias=-(2.0 ** -12), scale=2.0 ** -12
                )
                nc.vector.tensor_tensor(l16, n16, h12, ALU.subtract)
                a3 = a_hi.rearrange("p (c two) -> p c two", two=2)
                x3 = x_f.rearrange("p (c two) -> p c two", two=2)
                nc.vector.tensor_tensor(x3[:, :, 0], a3[:, :, 0], l16, ALU.add)
                nc.vector.tensor_tensor(x3[:, :, 1], a3[:, :, 1], h16, ALU.add)
            x_hi = xf_p.tile([P, D], F16, tag="xf", name=f"xhi{kc}")
            x_lo = xf_p.tile([P, D], F16, tag="xf", name=f"xlo{kc}")
            nc.scalar.copy(x_hi, x_f)
            nc.vector.tensor_tensor(x_lo, x_f, x_hi, ALU.subtract)
            # x_hi doubles as the AV (V) operand; spill for the pair gather.
            nc.sync.dma_start(kv_own[kc, :, 2 * D :], x_hi)
            for dc in range(NDC):
                for hl, x_h in ((0, x_hi), (1, x_lo)):
                    pt = paux.tile(
                        [P, P], F16, tag="paux", name=f"pt{kc}_{dc}_{hl}"
                    )
                    nc.tensor.transpose(
                        pt, x_h[:, dc * P : (dc + 1) * P], ident16
                    )
                    nc.vector.tensor_copy(
                        xt_j[:, hl, dc, sc * P : (sc + 1) * P], pt
                    )

        # kT / qT for these rows (own rows are both queries and keys).
        for do in range(NDC):
            for w_t, is_q in ((wk_t, False), (wq_t, True)):
                ps = p512.tile(
                    [P, JB], F32, tag="p512", name=f"ps{j}_{do}_{int(is_q)}"
                )
                nmm = len(HL) * NDC
                i = 0
                for dc in range(NDC):
                    for wh, xh_ in HL:
                        nc.tensor.matmul(
                            ps,
                            w_t[:, wh, dc, do * P : (do + 1) * P],
                            xt_j[:, xh_, dc, :],
                            start=(i == 0),
                            stop=(i == nmm - 1),
                        )
                        i += 1
                stg = out_p.tile(
                    [P, 2, JB], F16, tag="out", name=f"stg{j}_{do}_{int(is_q)}"
                )
                nc.scalar.copy(stg[:, 0, :], ps)
                nc.vector.tensor_tensor(
                    stg[:, 1, :], ps, stg[:, 0, :], ALU.subtract
                )
                if is_q:
                    for q2 in range(JB // SB):
                        qsb = j * (JB // SB) + q2
                        nc.sync.dma_start(
                            qt_d[qsb, :, :, do, :],
                            stg[:, :, q2 * SB : (q2 + 1) * SB],
                        )
                else:
                    for k4 in range(JB // P):
                        kc = j * (JB // P) + k4
                        nc.sync.dma_start(
                            kt_own_ap(kc)[:, :, do, :],
                            stg[:, :, k4 * P : (k4 + 1) * P],
                        )

    # ---------------- pair gather of keys/values ----------------
    nc.gpsimd.collective_compute(
        kind="AllGather", op=ALU.bypass, replica_groups=PAIR_GROUPS,
        ins=[kv_own], outs=[kv_all],
    )

    # ---------------- phase 2: attention ----------------
    for n in range(NSB):
        qt_n = med_p.tile([P, 2, NDC, SB], F16, tag="med", name=f"qt{n}")
        for dc in range(NDC):
            nc.sync.dma_start(qt_n[:, :, dc, :], qt_d[n, :, :, dc, :])

        st_t = stw_p.tile([P, NKC, SB], F32, tag="stw", name=f"st{n}")
        m_run = ms2_p.tile([P, SB], F32, tag="mrun", name=f"mrun{n}")

        for kc in range(NKC):
            kf_t = kf_p.tile([P, 2, NDC, P], F16, tag="kf", name=f"kf{n}_{kc}")
            nc.sync.dma_start(kf_t, kt_all_ap(kc))
            ps_s = pst.tile([P, SB], F32, tag="pst", name=f"pss{n}_{kc}")
            nmm = len(HL) * NDC
            i = 0
            for dc in range(NDC):
                for kh, qh in HL:
                    nc.tensor.matmul(
                        ps_s,
                        kf_t[:, kh, dc, :],
                        qt_n[:, qh, dc, :],
                        start=(i == 0),
                        stop=(i == nmm - 1),
                    )
                    i += 1
            # PSUM -> SBUF with the softmax scale applied (ACT, fp32).
            nc.scalar.mul(st_t[:, kc, :], ps_s, SCALE)
            # Running elementwise max over key chunks (kept unscaled; the
            # -SCALE broadcast constant rescales it to match st_t).
            if kc == 0:
                nc.vector.tensor_copy(m_run, ps_s)
            else:
                nc.vector.tensor_tensor(m_run, ps_s, m_run, ALU.max)

        # Column (per-query) max of m_run via PE transpose + DVE reduce.
        m_row = ms2_p.tile([1, SB], F32R, tag="mrow", name=f"mrow{n}")
        for h in range(SB // P):
            pt_m = paux.tile([P, P], F32, tag="paux", name=f"ptm{n}_{h}")
            nc.tensor.transpose(pt_m, m_run[:, h * P : (h + 1) * P], ident)
            m_col = ms2_p.tile([P, 1], F32, tag="mcol", name=f"mcol{n}_{h}")
            nc.vector.tensor_reduce(
                out=m_col, in_=pt_m, axis=AX.X, op=ALU.max
            )
            pt_r = paux.tile([1, P], F32, tag="paux", name=f"ptr{n}_{h}")
            nc.tensor.transpose(pt_r, m_col, ident)
            nc.vector.tensor_copy(m_row[:, h * P : (h + 1) * P], pt_r)

        # Broadcast -SCALE*max over the 128 key partitions.
        ps_m = paux.tile([P, SB], F32, tag="paux", name=f"psm{n}")
        nc.tensor.matmul(ps_m, negscale, m_row, start=True, stop=True)

        # s - m, then exp -> fp16 P written in place over the low half of
        # each fp32 chunk row (write offset trails read offset).
        p16 = st_t.bitcast(F16)  # [P, NKC, 2*SB]
        for kc in range(NKC):
            nc.vector.tensor_tensor(
                st_t[:, kc, :], st_t[:, kc, :], ps_m, ALU.add
            )
            nc.scalar.activation(p16[:, kc, :SB], st_t[:, kc, :], AF.Exp)

        # AV + row sums, streaming xv one d-half per pass.
        inv_t = ms2_p.tile([P, SB // P], F32, tag="inv", name=f"inv{n}")
        out_ts = [
            out_p.tile([P, D], F32, tag="out", name=f"o{n}_{qs}")
            for qs in range(SB // P)
        ]
        for dh in range(2):
            ps_av = [
                p512.tile([P, D // 2], F32, tag="p512", name=f"pav{n}_{dh}_{qs}")
                for qs in range(SB // P)
            ]
            if dh == 0:
                ps_sum = [
                    psm.tile([P, 1], F32, tag="psm", name=f"psum{n}_{qs}")
                    for qs in range(SB // P)
                ]
            for kc in range(NKC):
                xf_t = xf_p.tile([P, D // 2], F16, tag="xf", name=f"xa{n}_{dh}_{kc}")
                nc.sync.dma_start(
                    xf_t,
                    kv_all[
                        kc, :, 2 * D + dh * (D // 2) : 2 * D + (dh + 1) * (D // 2)
                    ],
                )
                for qs in range(SB // P):
                    pchunk = p16[:, kc, qs * P : (qs + 1) * P]
                    nc.tensor.matmul(
                        ps_av[qs],
                        pchunk,
                        xf_t,
                        start=(kc == 0),
                        stop=(kc == NKC - 1),
                    )
                    if dh == 0:
                        nc.tensor.matmul(
                            ps_sum[qs],
                            pchunk,
                            ones16,
                            start=(kc == 0),
                            stop=(kc == NKC - 1),
                        )
            for qs in range(SB // P):
                if dh == 0:
                    nc.vector.reciprocal(inv_t[:, qs : qs + 1], ps_sum[qs])
                nc.vector.tensor_scalar_mul(
                    out_ts[qs][:, dh * (D // 2) : (dh + 1) * (D // 2)],
                    ps_av[qs],
                    inv_t[:, qs : qs + 1],
                )
        for qs in range(SB // P):
            r0 = n * SB + qs * P
            q8 = q8_p.tile([P, D], U8, tag="q8", name=f"q8_{n}_{qs}")
            nc.scalar.activation(q8, out_ts[qs], AF.Copy, bias=OBIAS, scale=OSCALE)
            nc.sync.dma_start(out[r0 : r0 + P, :], q8)

    for p in reversed(ctx_pools):
        p.release()


# ---------------------------------------------------------------------------
# Host side: cached jit dispatch (built once), minimal transfers.
# ---------------------------------------------------------------------------

_CACHED = {}


def _runtime():
    if "rt" in _CACHED:
        return _CACHED["rt"]

    import jax
    from jax.sharding import Mesh, PartitionSpec, NamedSharding
    from jax.experimental.shard_map import shard_map

    nc = _build_module()
    bass2jax.install_neuronx_cc_hook()

    partition_name = nc.partition_id_tensor.name if nc.partition_id_tensor else None
    in_names, out_names, out_avals = [], [], []
    for alloc in nc.m.functions[0].allocations:
        if not isinstance(alloc, mybir.MemoryLocationSet):
            continue
        name = alloc.memorylocations[0].name
        if alloc.kind == "ExternalInput":
            if name != partition_name:
                in_names.append(name)
        elif alloc.kind == "ExternalOutput":
            out_names.append(name)
            out_avals.append(
                jax.core.ShapedArray(
                    tuple(alloc.tensor_shape), mybir.dt.np(alloc.dtype)
                )
            )
    bind_names = in_names + ([partition_name] if partition_name else [])

    def _body(*args):
        operands = list(args)
        if partition_name is not None:
            operands.append(bass2jax.partition_id_tensor())
        return tuple(
            bass2jax._bass_exec_p.bind(
                *operands,
                out_avals=tuple(out_avals),
                in_names=tuple(bind_names),
                out_names=tuple(out_names),
                lowering_input_output_aliases=(),
                sim_require_finite=True,
                sim_require_nnan=True,
                nc=nc,
            )
        )

    devices = jax.devices()[:NCORES]
    mesh = Mesh(np.asarray(devices), ("core",))
    spec = PartitionSpec("core")
    sharded = jax.jit(
        shard_map(
            _body, mesh=mesh,
            in_specs=(spec,) * len(in_names),
            out_specs=(spec,) * len(out_names),
            check_rep=False,
        ),
        keep_unused=True,
    )

    rt = types.SimpleNamespace(
        nc=nc,
        sharded=sharded,
        in_names=in_names,
        out_names=out_names,
        sharding=NamedSharding(mesh, spec),
        devices=devices,
        jax=jax,
        w_cache={},
    )
    _CACHED["rt"] = rt
    return rt


def _quant_x(xf):
    """x [N,D] f32 -> offset-binary XBITS-bit codes u [N, D] int32."""
    if XBITS <= 23:
        # Magic-number quantize: adding 2^23 forces RNE at integer ulp, and
        # the int32 view of the f32 is then 0x4B000000 | u (bit-exact vs
        # the rint path; verified). The magic's low nibble is 0, so shifts
        # and masks below 2^23 pass through unchanged.
        t = xf * np.float32(2.0 ** (XBITS - 4))
        t += np.float32(2.0 ** 23 + 2.0 ** (XBITS - 1))
        np.clip(t, np.float32(2.0 ** 23),
                np.float32(2.0 ** 23 + 2.0 ** XBITS - 1), out=t)
        return t.view(np.int32)
    t = xf * np.float32(2.0 ** (XBITS - 4))
    np.rint(t, out=t)
    u = t.astype(np.int32)
    del t
    u += 2 ** (XBITS - 1)
    np.clip(u, 0, 2 ** XBITS - 1, out=u)
    return u


def _w_dev(rt, w, key):
    """Device-cache a weight matrix keyed by content hash."""
    wc = np.ascontiguousarray(np.asarray(w, dtype=np.float32))
    h = hashlib.blake2b(wc.tobytes(), digest_size=16).digest()
    ent = rt.w_cache.get(key)
    if ent is not None and ent[0] == h:
        return ent[1]
    dev = rt.jax.device_put(wc, rt.sharding)
    rt.w_cache[key] = (h, dev)
    return dev


LAST_RESULTS = types.SimpleNamespace(exec_time_ns=None, results=None)


def kernel(x, Wq, Wk):
    rt = _runtime()
    jax = rt.jax
    # Pack and upload per-shard: device_put is async, so every shard's
    # packing after the first hides under the previous shards' uploads.
    x8 = np.ascontiguousarray(x, dtype=np.float32).reshape(NCORES, QH, D)
    xh_parts, xl_parts = [], []
    for c in range(NCORES):
        u = _quant_x(x8[c])
        xh_parts.append(jax.device_put((u >> (XBITS - 16)).astype(np.uint16),
                                       rt.devices[c]))
        if XBITS == 24:
            xl = u.astype(np.uint8)
        else:
            v = (u & 15).astype(np.uint8)
            xl = v[:, 0::2] | (v[:, 1::2] << 4)
        xl_parts.append(jax.device_put(xl, rt.devices[c]))
    xh_d = jax.make_array_from_single_device_arrays(
        (NCORES * QH, D), rt.sharding, xh_parts)
    xl_d = jax.make_array_from_single_device_arrays(
        (NCORES * QH, D if XBITS == 24 else D // 2), rt.sharding, xl_parts)
    wq_d = _w_dev(rt, Wq, "wq")
    wk_d = _w_dev(rt, Wk, "wk")
    args = {"xh": xh_d, "xl": xl_d, "wq": wq_d, "wk": wk_d}
    outs = rt.sharded(*[args[n] for n in rt.in_names])
    # Fetch + dequantize per shard; async host copies keep the tunnel busy
    # while earlier shards dequantize.
    shards = sorted(outs[0].addressable_shards, key=lambda s: s.index[0].start or 0)
    for s in shards:
        try:
            s.data.copy_to_host_async()
        except Exception:
            pass
    o = np.empty((NCORES, QH, D), np.float32)
    for i, s in enumerate(shards):
        o_u8 = np.asarray(s.data)
        oc = o[i]
        np.subtract(o_u8, np.float32(OBIAS), out=oc)
        oc *= 1.0 / OSCALE
    return o.reshape(B, S, D)


# revision 47
# speedup vs baseline: 1.2593x; 1.2593x over previous
"""Classical self-attention on 8 Trainium2 NeuronCores.

out = softmax((x Wq)(x Wk)^T / sqrt(D)) @ x   with x:[4,4096,1024] f32.

End-to-end wall time on this setup is dominated by the axon tunnel
(~38 MB/s), so the kernel is organized around minimizing host<->device
bytes; device compute (a few ms) is a rounding error by comparison.

Distribution: 8 shards = (batch, seq-half). Each core receives ONLY its
own 2048 rows of x, quantized to 20-bit fixed point over [-8,8) (uint16
plane with the top 16 bits + packed-nibble plane, 5 MB/core -> 40 MB
total instead of 128 MB f32-duplicated) plus a 1/8 row-slice of Wq/Wk
(1 MB/core, device-cached across calls by content hash). On device:
  - 8-way AllGather reassembles full Wq/Wk from the slices;
  - each core reconstructs its own x rows exactly (the 24-bit unpack is
    exact in f32), projects them to kT (fp16 hi/lo) and qT, and spills a
    fp16 copy of x (the V operand);
  - a pair AllGather (cores 2b, 2b+1) exchanges kT and the fp16 V copy
    over NeuronLink, giving every core all 4096 keys/values of its batch
    in rank order (softmax is permutation-invariant over keys, and the
    AV matmul uses the same gathered row order, so order drops out);
  - flash-style attention over the gathered keys, per 256-query block;
  - the output is quantized to uint8 (step 1/20, RNE) on device: 2 MB/core
    down instead of 8 MB. Dequantized on host; quant error ~0.025 abs vs a
    2e-2*absmax (~0.11) budget.

Precision: softmax logits have std ~1000, so the score path runs as fp16
hi/lo decompositions (a = hi + lo; a*b = ah*bh + ah*bl + al*bh, the al*bl
term dropped) carrying ~22 mantissa bits at full PE rate. The 20-bit x
quantization (step 2^-16) gives ~6e-3 rms logit error -> ~0.03 extra
absmax output error on near-tie softmax rows; with the output
quantization this totals ~9.5e-3 relative, 2x under the 2e-2 gate. The
unpack on device is exact in f32 (all power-of-two scales; the odd
nibble is recovered with the round-to-nearest f32->u8 conversion). The
AV matmul runs plain fp16.

Host path: the jit-wrapped bass_exec call is built ONCE and cached
(rebuilding it per call costs ~3s of XLA retrace/recompile), outputs are
custom-call results (no donated zero buffers to upload), packing and
uploads are pipelined per shard (device_put is async, so only the first
shard's packing is exposed), and the output fetch is pipelined with the
host-side dequantization. Per call: ~40 MB up + ~16 MB down at the
tunnel's ~35-50 MB/s, ~115 ms dispatch+execute (of which ~86 ms is the
axon round-trip floor).
"""

import hashlib
import types

import numpy as np

import concourse.bass as bass  # noqa: F401  (keeps the usual import env)
import concourse.mybir as mybir
import concourse.tile as tile
import concourse.bass2jax as bass2jax
from concourse import bacc
from concourse.masks import make_identity

# Problem constants (hardcoded: kernel.py must be self-contained).
B, S, D = 4, 4096, 1024
NCORES = 8
QH = S // 2            # rows (queries) per core
P = 128
NDC = D // P           # 8 d-chunks
SB = 256               # query superblock
NSB = QH // SB         # 8 superblocks per core
NKC = S // P           # 32 gathered key chunks
NOC = QH // P          # 16 own row chunks
JB = 512               # proj seq-block
NJ = QH // JB          # 4 own j-blocks
SCALE = 1.0 / float(np.sqrt(np.float32(D)))
HL = ((0, 0), (0, 1), (1, 0))  # hi/lo term pairs (lhs_split, rhs_split)

# Fixed point for x over [-8, 8). XBITS=24: u16 hi plane + u8 lo plane.
# XBITS=20: u16 plane (top 16 bits) + packed-nibble plane (2 els/byte).
XBITS = 20
# Near-max mask threshold (logit units) and purity bound for host-side
# pure-argmax reconstruction; rows failing either are fetched as u8.
# TAU must exceed the f32r max-broadcast error (~0.46 logits at |m|~3800):
# the argmax then always passes, and any second key inside TAU carries
# >= ~0.12 softmax weight, so the RHO_PURE check demotes such rows anyway.
TAU = 1.5
RHO_PURE = 1e-3
TIE_PAD = 128          # fetched tie rows per core (measured max 46 on the
                       # reference input; overflow falls back to full fetch)
# uint8 output quantization: u8 = round(out*OSCALE + 128). Range +-6.4
# comfortably covers |out| <= absmax(x) ~ 5.45 (outputs are convex
# combinations of x rows) while giving a 25% finer step than +-8.
OSCALE = 20.0
OBIAS = 128.0

F32 = mybir.dt.float32
F32R = mybir.dt.float32r
F16 = mybir.dt.float16
U16 = mybir.dt.uint16
U8 = mybir.dt.uint8
ALU = mybir.AluOpType
AX = mybir.AxisListType
AF = mybir.ActivationFunctionType

PAIR_GROUPS = [[0, 1], [2, 3], [4, 5], [6, 7]]
ALL_GROUP = [[0, 1, 2, 3, 4, 5, 6, 7]]


def _build_module():
    nc = bacc.Bacc(
        trn_type="TRN2",
        target_bir_lowering=False,
        debug=False,
        enable_asserts=False,
        num_devices=NCORES,
    )
    xh = nc.dram_tensor("xh", [QH, D], U16, kind="ExternalInput").ap()
    xl_cols = D if XBITS == 24 else D // 2
    xl = nc.dram_tensor("xl", [QH, xl_cols], U8, kind="ExternalInput").ap()
    wq = nc.dram_tensor("wq", [P, D], F32, kind="ExternalInput").ap()
    wk = nc.dram_tensor("wk", [P, D], F32, kind="ExternalInput").ap()
    out = nc.dram_tensor("out", [QH, D], U8, kind="ExternalOutput").ap()
    # Per-query row stats: [argmax index numerator, near-max count,
    # softmax row sum, max softmax term]. Lets the host reconstruct
    # pure-argmax rows from its own exact x and fetch only near-tie rows
    # of `out`.
    aux = nc.dram_tensor("aux", [QH, 4], F32, kind="ExternalOutput").ap()

    with tile.TileContext(nc) as tc:
        _emit(tc, nc, xh, xl, wq, wk, out, aux)
    nc.compile()
    return nc


def _emit(tc, nc, xh, xl, wq, wk, out, aux):
    ctx_pools = []

    def pool(**kw):
        p = tc.alloc_tile_pool(**kw)
        ctx_pools.append(p)
        return p

    # SBUF pools (per-partition KB in comments).
    stw_p = pool(name="stw", bufs=2)          # 2 x 32KB slots (W16 / ST shared)
    med_p = pool(name="med", bufs=2)          # 2 x 16KB (xt_j / qT)
    xs_p = pool(name="xs", bufs=3)            # 3 x 4KB (f32 chunk temps)
    xf_p = pool(name="xf", bufs=4)            # 4 x 2KB (fp16 staging/stream)
    xq_p = pool(name="xq", bufs=3)            # 3 x 2KB (u16/u8 plane loads)
    kf_p = pool(name="kf", bufs=3)            # 3 x 4KB (kT stream)
    out_p = pool(name="outp", bufs=2)         # 2 x 4KB (out f32 / spill staging)
    q8_p = pool(name="q8", bufs=2)            # 2 x 1KB (u8 out staging)
    msc_p = pool(name="msc", bufs=1)          # constants
    ms2_p = pool(name="ms2", bufs=2)          # rotating smalls

    # PSUM pools (8 banks total).
    p512 = pool(name="p512", bufs=2, space="PSUM")   # proj + AV [128,512]
    pst = pool(name="pst", bufs=2, space="PSUM")     # ST chunks [128,256]
    paux = pool(name="paux", bufs=2, space="PSUM")   # transposes / bcast
    psm = pool(name="psm", bufs=2, space="PSUM")     # row-sum accumulators

    # DRAM scratch.
    dram = pool(name="dram", bufs=1, space="DRAM")
    w_b = dram.tile([2, P, D], F32, tag="wb", name="w_b")
    w_all = dram.tile([NCORES, 2, P, D], F32, tag="wall", name="w_all")
    # kT (2*NDC*P = 2048 cols) and the fp16 V copy (D = 1024 cols) share one
    # tensor per key chunk so the pair exchange is a single AllGather.
    kv_own = dram.tile([NOC, P, 3 * D], F16, tag="kvo", name="kv_own")
    kv_all = dram.tile([NKC, P, 3 * D], F16, tag="kva", name="kv_all")
    qt_d = dram.tile([NSB, P, 2, NDC, SB], F16, tag="qtd", name="qt_d")

    def kt_own_ap(kc):
        return kv_own[kc, :, : 2 * D].rearrange(
            "p (hl dc k) -> p hl dc k", hl=2, dc=NDC
        )

    def kt_all_ap(kc):
        return kv_all[kc, :, : 2 * D].rearrange(
            "p (hl dc k) -> p hl dc k", hl=2, dc=NDC
        )

    # Constants.
    ident = msc_p.tile([P, P], F32, tag="ident", name="ident")
    make_identity(nc, ident)
    ident16 = msc_p.tile([P, P], F16, tag="ident16", name="ident16")
    nc.vector.tensor_copy(ident16, ident)
    negs32 = msc_p.tile([1, P], F32, tag="negs32", name="negs32")
    nc.gpsimd.memset(negs32, -SCALE)
    negscale = msc_p.tile([1, P], F32R, tag="negscale", name="negscale")
    nc.vector.tensor_copy(negscale, negs32)
    ones32 = msc_p.tile([P, 1], F32, tag="ones32", name="ones32")
    nc.gpsimd.memset(ones32, 1.0)
    ones16 = msc_p.tile([P, 1], F16, tag="ones16", name="ones16")
    nc.vector.tensor_copy(ones16, ones32)
    # Row-stat matmul RHS: per key chunk kc, cols [3kc, 3kc+1, 3kc+2] =
    # [1, partition index, kc] so one [P,3] matmul per (kc, qs) accumulates
    # [row sum, sum(P*local key), sum(P*chunk idx)] (all fp16-exact ints).
    iota32 = msc_p.tile([P, 1], mybir.dt.int32, tag="iota32", name="iota32")
    nc.gpsimd.iota(iota32, pattern=[[0, 1]], base=0, channel_multiplier=1)
    iotaf = msc_p.tile([P, 1], F32, tag="iotaf", name="iotaf")
    nc.vector.tensor_copy(iotaf, iota32)
    rks = msc_p.tile([P, 3 * NKC], F16, tag="rks", name="rks")
    for kc in range(NKC):
        nc.gpsimd.memset(rks[:, 3 * kc : 3 * kc + 1], 1.0)
        nc.vector.tensor_copy(rks[:, 3 * kc + 1 : 3 * kc + 2], iotaf)
        nc.gpsimd.memset(rks[:, 3 * kc + 2 : 3 * kc + 3], float(kc))

    # ---------------- phase 0: weight gather + split ----------------
    nc.sync.dma_start(w_b[0], wq)
    nc.sync.dma_start(w_b[1], wk)
    nc.gpsimd.collective_compute(
        kind="AllGather", op=ALU.bypass, replica_groups=ALL_GROUP,
        ins=[w_b], outs=[w_all],
    )

    # Weights as fp16 hi/lo: w16[:, hl, din_chunk, dout].
    wq_t = stw_p.tile([P, 2, NDC, D], F16, tag="stw", name="wq_t")
    wk_t = stw_p.tile([P, 2, NDC, D], F16, tag="stw", name="wk_t")
    for w_dst, wi, wn in ((wq_t, 0, "q"), (wk_t, 1, "k")):
        for i in range(NDC):
            w_in = xs_p.tile([P, D], F32, tag="xs", name=f"w{wn}in{i}")
            nc.sync.dma_start(w_in, w_all[i, wi])
            nc.scalar.copy(w_dst[:, 0, i, :], w_in)
            nc.vector.tensor_tensor(
                w_dst[:, 1, i, :], w_in, w_dst[:, 0, i, :], ALU.subtract
            )

    # ---------------- phase 1: unpack + projections (own rows) ----------
    for j in range(NJ):
        xt_j = med_p.tile([P, 2, NDC, JB], F16, tag="med", name=f"xt{j}")
        for sc in range(JB // P):
            kc = j * (JB // P) + sc
            row0 = kc * P
            xh_t = xq_p.tile([P, D], U16, tag="xq", name=f"xh{kc}")
            nc.sync.dma_start(xh_t, xh[row0 : row0 + P, :])
            a_hi = xs_p.tile([P, D], F32, tag="xs", name=f"ah{kc}")
            x_f = xs_p.tile([P, D], F32, tag="xs", name=f"xr{kc}")
            if XBITS == 24:
                # exact unpack: x = uh*2^-12 - 8 + ul*2^-20
                xl_t = xq_p.tile([P, D], U8, tag="xq", name=f"xl{kc}")
                nc.sync.dma_start(xl_t, xl[row0 : row0 + P, :])
                a_lo = xs_p.tile([P, D], F32, tag="xs", name=f"al{kc}")
                nc.scalar.activation(
                    a_hi, xh_t, AF.Copy, bias=-8.0, scale=2.0 ** -12
                )
                nc.scalar.activation(
                    a_lo, xl_t, AF.Copy, bias=0.0, scale=2.0 ** -20
                )
                nc.vector.tensor_tensor(x_f, a_hi, a_lo, ALU.add)
            else:
                # x = A*2^-12 - 8 + n_even*2^-16 (even cols) / n_odd*2^-16
                # (odd cols), nibbles n packed two-per-byte in xl. The odd
                # nibble is recovered as h' = rne(n/16 + 1 - 0.46875) =
                # n_odd + 1 via the (round-to-nearest) f32->u8 conversion;
                # every step below is exact in f32.
                xl_t = xq_p.tile([P, D // 2], U8, tag="xq", name=f"xl{kc}")
                nc.sync.dma_start(xl_t, xl[row0 : row0 + P, :])
                nc.scalar.activation(
                    a_hi, xh_t, AF.Copy, bias=-8.0, scale=2.0 ** -12
                )
                n16 = xs_p.tile([P, D // 2], F32, tag="nib", name=f"n16_{kc}")
                hp = xq_p.tile([P, D // 2], U8, tag="hp", name=f"hp{kc}")
                h16 = xs_p.tile([P, D // 2], F32, tag="nib", name=f"h16_{kc}")
                h12 = xs_p.tile([P, D // 2], F32, tag="nib", name=f"h12_{kc}")
                l16 = xs_p.tile([P, D // 2], F32, tag="nib", name=f"l16_{kc}")
                nc.scalar.activation(
                    n16, xl_t, AF.Copy, bias=0.0, scale=2.0 ** -16
                )
                nc.scalar.activation(
                    hp, xl_t, AF.Copy, bias=1.0 - 0.46875, scale=2.0 ** -4
                )
                nc.scalar.activation(
                    h16, hp, AF.Copy, bias=-(2.0 ** -16), scale=2.0 ** -16
                )
                nc.scalar.activation(
                    h12, hp, AF.Copy, bias=-(2.0 ** -12), scale=2.0 ** -12
                )
                nc.vector.tensor_tensor(l16, n16, h12, ALU.subtract)
                a3 = a_hi.rearrange("p (c two) -> p c two", two=2)
                x3 = x_f.rearrange("p (c two) -> p c two", two=2)
                nc.vector.tensor_tensor(x3[:, :, 0], a3[:, :, 0], l16, ALU.add)
                nc.vector.tensor_tensor(x3[:, :, 1], a3[:, :, 1], h16, ALU.add)
            x_hi = xf_p.tile([P, D], F16, tag="xf", name=f"xhi{kc}")
            x_lo = xf_p.tile([P, D], F16, tag="xf", name=f"xlo{kc}")
            nc.scalar.copy(x_hi, x_f)
            nc.vector.tensor_tensor(x_lo, x_f, x_hi, ALU.subtract)
            # x_hi doubles as the AV (V) operand; spill for the pair gather.
            nc.sync.dma_start(kv_own[kc, :, 2 * D :], x_hi)
            for dc in range(NDC):
                for hl, x_h in ((0, x_hi), (1, x_lo)):
                    pt = paux.tile(
                        [P, P], F16, tag="paux", name=f"pt{kc}_{dc}_{hl}"
                    )
                    nc.tensor.transpose(
                        pt, x_h[:, dc * P : (dc + 1) * P], ident16
                    )
                    nc.vector.tensor_copy(
                        xt_j[:, hl, dc, sc * P : (sc + 1) * P], pt
                    )

        # kT / qT for these rows (own rows are both queries and keys).
        for do in range(NDC):
            for w_t, is_q in ((wk_t, False), (wq_t, True)):
                ps = p512.tile(
                    [P, JB], F32, tag="p512", name=f"ps{j}_{do}_{int(is_q)}"
                )
                nmm = len(HL) * NDC
                i = 0
                for dc in range(NDC):
                    for wh, xh_ in HL:
                        nc.tensor.matmul(
                            ps,
                            w_t[:, wh, dc, do * P : (do + 1) * P],
                            xt_j[:, xh_, dc, :],
                            start=(i == 0),
                            stop=(i == nmm - 1),
                        )
                        i += 1
                stg = out_p.tile(
                    [P, 2, JB], F16, tag="out", name=f"stg{j}_{do}_{int(is_q)}"
                )
                nc.scalar.copy(stg[:, 0, :], ps)
                nc.vector.tensor_tensor(
                    stg[:, 1, :], ps, stg[:, 0, :], ALU.subtract
                )
                if is_q:
                    for q2 in range(JB // SB):
                        qsb = j * (JB // SB) + q2
                        nc.sync.dma_start(
                            qt_d[qsb, :, :, do, :],
                            stg[:, :, q2 * SB : (q2 + 1) * SB],
                        )
                else:
                    for k4 in range(JB // P):
                        kc = j * (JB // P) + k4
                        nc.sync.dma_start(
                            kt_own_ap(kc)[:, :, do, :],
                            stg[:, :, k4 * P : (k4 + 1) * P],
                        )

    # ---------------- pair gather of keys/values ----------------
    nc.gpsimd.collective_compute(
        kind="AllGather", op=ALU.bypass, replica_groups=PAIR_GROUPS,
        ins=[kv_own], outs=[kv_all],
    )

    # ---------------- phase 2: attention ----------------
    for n in range(NSB):
        qt_n = med_p.tile([P, 2, NDC, SB], F16, tag="med", name=f"qt{n}")
        for dc in range(NDC):
            nc.sync.dma_start(qt_n[:, :, dc, :], qt_d[n, :, :, dc, :])

        st_t = stw_p.tile([P, NKC, SB], F32, tag="stw", name=f"st{n}")
        m_run = ms2_p.tile([P, SB], F32, tag="mrun", name=f"mrun{n}")

        for kc in range(NKC):
            kf_t = kf_p.tile([P, 2, NDC, P], F16, tag="kf", name=f"kf{n}_{kc}")
            nc.sync.dma_start(kf_t, kt_all_ap(kc))
            ps_s = pst.tile([P, SB], F32, tag="pst", name=f"pss{n}_{kc}")
            nmm = len(HL) * NDC
            i = 0
            for dc in range(NDC):
                for kh, qh in HL:
                    nc.tensor.matmul(
                        ps_s,
                        kf_t[:, kh, dc, :],
                        qt_n[:, qh, dc, :],
                        start=(i == 0),
                        stop=(i == nmm - 1),
                    )
                    i += 1
            # PSUM -> SBUF with the softmax scale applied (ACT, fp32).
            nc.scalar.mul(st_t[:, kc, :], ps_s, SCALE)
            # Running elementwise max over key chunks (kept unscaled; the
            # -SCALE broadcast constant rescales it to match st_t).
            if kc == 0:
                nc.vector.tensor_copy(m_run, ps_s)
            else:
                nc.vector.tensor_tensor(m_run, ps_s, m_run, ALU.max)

        # Column (per-query) max of m_run via PE transpose + DVE reduce.
        m_row = ms2_p.tile([1, SB], F32R, tag="mrow", name=f"mrow{n}")
        for h in range(SB // P):
            pt_m = paux.tile([P, P], F32, tag="paux", name=f"ptm{n}_{h}")
            nc.tensor.transpose(pt_m, m_run[:, h * P : (h + 1) * P], ident)
            m_col = ms2_p.tile([P, 1], F32, tag="mcol", name=f"mcol{n}_{h}")
            nc.vector.tensor_reduce(
                out=m_col, in_=pt_m, axis=AX.X, op=ALU.max
            )
            pt_r = paux.tile([1, P], F32, tag="paux", name=f"ptr{n}_{h}")
            nc.tensor.transpose(pt_r, m_col, ident)
            nc.vector.tensor_copy(m_row[:, h * P : (h + 1) * P], pt_r)

        # Broadcast -SCALE*max over the 128 key partitions.
        ps_m = paux.tile([P, SB], F32, tag="paux", name=f"psm{n}")
        nc.tensor.matmul(ps_m, negscale, m_row, start=True, stop=True)

        # s - m, then exp -> fp16 P written in place over the low half of
        # each fp32 chunk row (write offset trails read offset).
        p16 = st_t.bitcast(F16)  # [P, NKC, 2*SB]
        m2_run = ms2_p.tile([P, SB], F32, tag="m2run", name=f"m2run{n}")
        # Row stats: col 0 = softmax row sum (accumulated in the AV loop);
        # cols 1:4 = [count, sum(mask*i), sum(mask*kc)] of near-max keys
        # (mask = logit within TAU of the row max), exact small integers.
        ps4 = [
            psm.tile([P, 4], F32, tag="psm", name=f"ps4_{n}_{qs}")
            for qs in range(SB // P)
        ]
        for kc in range(NKC):
            nc.vector.tensor_tensor(
                st_t[:, kc, :], st_t[:, kc, :], ps_m, ALU.add
            )
            mk = ms2_p.tile([P, SB], F16, tag="mk", name=f"mk{n}_{kc}")
            nc.vector.tensor_scalar(
                mk, st_t[:, kc, :], -TAU, None, ALU.is_ge
            )
            for qs in range(SB // P):
                nc.tensor.matmul(
                    ps4[qs][:, 1:4],
                    mk[:, qs * P : (qs + 1) * P],
                    rks[:, 3 * kc : 3 * kc + 3],
                    start=(kc == 0),
                    stop=(kc == NKC - 1),
                )
            nc.scalar.activation(p16[:, kc, :SB], st_t[:, kc, :], AF.Exp)
            if kc == 0:
                nc.vector.tensor_copy(m2_run, p16[:, kc, :SB])
            else:
                nc.vector.tensor_tensor(
                    m2_run, p16[:, kc, :SB], m2_run, ALU.max
                )

        # Per-query max softmax term (transpose + reduce, like the m path).
        m2c = []
        for qs in range(SB // P):
            ptm2 = paux.tile([P, P], F32, tag="paux", name=f"ptm2{n}_{qs}")
            nc.tensor.transpose(
                ptm2, m2_run[:, qs * P : (qs + 1) * P], ident
            )
            mc = ms2_p.tile([P, 1], F32, tag="m2c", name=f"m2c{n}_{qs}")
            nc.vector.tensor_reduce(out=mc, in_=ptm2, axis=AX.X, op=ALU.max)
            m2c.append(mc)

        # AV + row sums, streaming xv one d-half per pass.
        inv_t = ms2_p.tile([P, SB // P], F32, tag="inv", name=f"inv{n}")
        out_ts = [
            out_p.tile([P, D], F32, tag="out", name=f"o{n}_{qs}")
            for qs in range(SB // P)
        ]
        for dh in range(2):
            ps_av = [
                p512.tile([P, D // 2], F32, tag="p512", name=f"pav{n}_{dh}_{qs}")
                for qs in range(SB // P)
            ]

            for kc in range(NKC):
                xf_t = xf_p.tile([P, D // 2], F16, tag="xf", name=f"xa{n}_{dh}_{kc}")
                nc.sync.dma_start(
                    xf_t,
                    kv_all[
                        kc, :, 2 * D + dh * (D // 2) : 2 * D + (dh + 1) * (D // 2)
                    ],
                )
                for qs in range(SB // P):
                    pchunk = p16[:, kc, qs * P : (qs + 1) * P]
                    nc.tensor.matmul(
                        ps_av[qs],
                        pchunk,
                        xf_t,
                        start=(kc == 0),
                        stop=(kc == NKC - 1),
                    )
                    if dh == 0:
                        nc.tensor.matmul(
                            ps4[qs][:, 0:1],
                            pchunk,
                            ones16,
                            start=(kc == 0),
                            stop=(kc == NKC - 1),
                        )
            for qs in range(SB // P):
                if dh == 0:
                    nc.vector.reciprocal(
                        inv_t[:, qs : qs + 1], ps4[qs][:, 0:1]
                    )
                    # aux block: [idx numerator, count, row sum, maxP]
                    ax = ms2_p.tile([P, 4], F32, tag="aux", name=f"ax{n}_{qs}")
                    nc.scalar.activation(
                        ax[:, 0:1], ps4[qs][:, 3:4], AF.Copy,
                        bias=0.0, scale=128.0,
                    )
                    nc.vector.tensor_tensor(
                        ax[:, 0:1], ps4[qs][:, 2:3], ax[:, 0:1], ALU.add
                    )
                    nc.scalar.copy(ax[:, 1:2], ps4[qs][:, 1:2])
                    nc.scalar.copy(ax[:, 2:3], ps4[qs][:, 0:1])
                    nc.vector.tensor_copy(ax[:, 3:4], m2c[qs])
                    r0a = n * SB + qs * P
                    nc.sync.dma_start(aux[r0a : r0a + P, :], ax)
                nc.vector.tensor_scalar_mul(
                    out_ts[qs][:, dh * (D // 2) : (dh + 1) * (D // 2)],
                    ps_av[qs],
                    inv_t[:, qs : qs + 1],
                )
        for qs in range(SB // P):
            r0 = n * SB + qs * P
            q8 = q8_p.tile([P, D], U8, tag="q8", name=f"q8_{n}_{qs}")
            nc.scalar.activation(q8, out_ts[qs], AF.Copy, bias=OBIAS, scale=OSCALE)
            nc.sync.dma_start(out[r0 : r0 + P, :], q8)

    for p in reversed(ctx_pools):
        p.release()


# ---------------------------------------------------------------------------
# Host side: cached jit dispatch (built once), minimal transfers.
# ---------------------------------------------------------------------------

_CACHED = {}


def _runtime():
    if "rt" in _CACHED:
        return _CACHED["rt"]

    import jax
    from jax.sharding import Mesh, PartitionSpec, NamedSharding
    from jax.experimental.shard_map import shard_map

    nc = _build_module()
    bass2jax.install_neuronx_cc_hook()

    partition_name = nc.partition_id_tensor.name if nc.partition_id_tensor else None
    in_names, out_names, out_avals = [], [], []
    for alloc in nc.m.functions[0].allocations:
        if not isinstance(alloc, mybir.MemoryLocationSet):
            continue
        name = alloc.memorylocations[0].name
        if alloc.kind == "ExternalInput":
            if name != partition_name:
                in_names.append(name)
        elif alloc.kind == "ExternalOutput":
            out_names.append(name)
            out_avals.append(
                jax.core.ShapedArray(
                    tuple(alloc.tensor_shape), mybir.dt.np(alloc.dtype)
                )
            )
    bind_names = in_names + ([partition_name] if partition_name else [])

    def _body(*args):
        operands = list(args)
        if partition_name is not None:
            operands.append(bass2jax.partition_id_tensor())
        return tuple(
            bass2jax._bass_exec_p.bind(
                *operands,
                out_avals=tuple(out_avals),
                in_names=tuple(bind_names),
                out_names=tuple(out_names),
                lowering_input_output_aliases=(),
                sim_require_finite=True,
                sim_require_nnan=True,
                nc=nc,
            )
        )

    devices = jax.devices()[:NCORES]
    mesh = Mesh(np.asarray(devices), ("core",))
    spec = PartitionSpec("core")
    sharded = jax.jit(
        shard_map(
            _body, mesh=mesh,
            in_specs=(spec,) * len(in_names),
            out_specs=(spec,) * len(out_names),
            check_rep=False,
        ),
        keep_unused=True,
    )

    def _gather_local(a, i):
        return a[i[0]]

    gather_jit = jax.jit(
        shard_map(
            _gather_local, mesh=mesh,
            in_specs=(spec, spec), out_specs=spec, check_rep=False,
        )
    )

    rt = types.SimpleNamespace(
        nc=nc,
        sharded=sharded,
        in_names=in_names,
        out_names=out_names,
        sharding=NamedSharding(mesh, spec),
        devices=devices,
        jax=jax,
        gather=gather_jit,
        w_cache={},
    )
    _CACHED["rt"] = rt
    return rt


def _quant_x(xf):
    """x [N,D] f32 -> offset-binary XBITS-bit codes u [N, D] int32."""
    if XBITS <= 23:
        # Magic-number quantize: adding 2^23 forces RNE at integer ulp, and
        # the int32 view of the f32 is then 0x4B000000 | u (bit-exact vs
        # the rint path; verified). The magic's low nibble is 0, so shifts
        # and masks below 2^23 pass through unchanged.
        t = xf * np.float32(2.0 ** (XBITS - 4))
        t += np.float32(2.0 ** 23 + 2.0 ** (XBITS - 1))
        np.clip(t, np.float32(2.0 ** 23),
                np.float32(2.0 ** 23 + 2.0 ** XBITS - 1), out=t)
        return t.view(np.int32)
    t = xf * np.float32(2.0 ** (XBITS - 4))
    np.rint(t, out=t)
    u = t.astype(np.int32)
    del t
    u += 2 ** (XBITS - 1)
    np.clip(u, 0, 2 ** XBITS - 1, out=u)
    return u


def _w_dev(rt, w, key):
    """Device-cache a weight matrix keyed by content hash."""
    wc = np.ascontiguousarray(np.asarray(w, dtype=np.float32))
    h = hashlib.blake2b(wc.tobytes(), digest_size=16).digest()
    ent = rt.w_cache.get(key)
    if ent is not None and ent[0] == h:
        return ent[1]
    dev = rt.jax.device_put(wc, rt.sharding)
    rt.w_cache[key] = (h, dev)
    return dev


LAST_RESULTS = types.SimpleNamespace(exec_time_ns=None, results=None)


def kernel(x, Wq, Wk):
    rt = _runtime()
    jax = rt.jax
    # Pack and upload per-shard: device_put is async, so every shard's
    # packing after the first hides under the previous shards' uploads.
    x8 = np.ascontiguousarray(x, dtype=np.float32).reshape(NCORES, QH, D)
    xh_parts, xl_parts = [], []
    for c in range(NCORES):
        u = _quant_x(x8[c])
        xh_parts.append(jax.device_put((u >> (XBITS - 16)).astype(np.uint16),
                                       rt.devices[c]))
        if XBITS == 24:
            xl = u.astype(np.uint8)
        else:
            v = (u & 15).astype(np.uint8)
            xl = v[:, 0::2] | (v[:, 1::2] << 4)
        xl_parts.append(jax.device_put(xl, rt.devices[c]))
    xh_d = jax.make_array_from_single_device_arrays(
        (NCORES * QH, D), rt.sharding, xh_parts)
    xl_d = jax.make_array_from_single_device_arrays(
        (NCORES * QH, D if XBITS == 24 else D // 2), rt.sharding, xl_parts)
    wq_d = _w_dev(rt, Wq, "wq")
    wk_d = _w_dev(rt, Wk, "wk")
    args = {"xh": xh_d, "xl": xl_d, "wq": wq_d, "wk": wk_d}
    outs = rt.sharded(*[args[n] for n in rt.in_names])
    om = dict(zip(rt.out_names, outs))
    # Queue the aux host-copy behind the execute so classification data
    # streams back the moment the kernel finishes.
    try:
        for s in om["aux"].addressable_shards:
            s.data.copy_to_host_async()
    except Exception:
        pass
    try:
        o = _fetch_sparse(rt, om, x8)
    except Exception:
        o = _fetch_full(om["out"])
    return o.reshape(B, S, D)


def _fetch_full(out_arr):
    """Fetch the whole u8 output tensor and dequantize (fallback path)."""
    shards = sorted(
        out_arr.addressable_shards, key=lambda s: s.index[0].start or 0
    )
    for s in shards:
        try:
            s.data.copy_to_host_async()
        except Exception:
            pass
    o = np.empty((NCORES, QH, D), np.float32)
    for i, s in enumerate(shards):
        o_u8 = np.asarray(s.data)
        oc = o[i]
        np.subtract(o_u8, np.float32(OBIAS), out=oc)
        oc *= 1.0 / OSCALE
    return o


def _fetch_sparse(rt, om, x8):
    """Reconstruct pure-argmax rows from exact x; fetch only near-tie rows.

    aux cols: [idx numerator, near-max count, softmax row sum, maxP]. A row
    is pure iff exactly one key sits within TAU of the row max AND the
    non-max softmax mass is < RHO_PURE (then |out - x[idx]| <= ~2e-2*RHO).
    Any anomaly (count!=1, non-integer stats, overflow) demotes the row to
    the fetched u8 path; bucket overflow falls back to a full fetch.
    """
    aux = np.asarray(om["aux"]).reshape(NCORES * QH, 4).astype(np.float64)
    idxn, cnt, ssum, maxp = aux[:, 0], aux[:, 1], aux[:, 2], aux[:, 3]
    idx = np.rint(idxn)
    pure = (
        np.isfinite(aux).all(axis=1)
        & (np.abs(cnt - 1.0) < 0.01)
        & (np.abs(idxn - idx) < 0.01)
        & (idx >= 0) & (idx < S)
        & (maxp > 0.1)
        & (ssum - maxp <= RHO_PURE * maxp)
    )
    tie = np.nonzero(~pure)[0]
    percore = [tie[(tie >= c * QH) & (tie < (c + 1) * QH)] - c * QH
               for c in range(NCORES)]
    if max(len(t) for t in percore) > TIE_PAD:
        return _fetch_full(om["out"])

    # Dispatch the tie-row gather (one sharded jit over the device-resident
    # u8 output), then fill pure rows from exact x while it runs.
    pads = np.zeros((NCORES, TIE_PAD), np.int32)
    for c in range(NCORES):
        pads[c, : len(percore[c])] = percore[c]
    gathered = rt.gather(om["out"], pads)

    o = np.empty((NCORES * QH, D), np.float32)
    xflat = x8.reshape(NCORES * QH, D)  # == x.reshape(B*S, D)
    g = np.arange(NCORES * QH)
    src = (g // S) * S + np.clip(idx.astype(np.int64), 0, S - 1)
    o[pure] = xflat[src[pure]]

    gat = np.asarray(gathered).reshape(NCORES, TIE_PAD, D)
    for c in range(NCORES):
        loc = percore[c]
        if len(loc) == 0:
            continue
        rows = gat[c, : len(loc)].astype(np.float32)
        rows -= OBIAS
        rows *= 1.0 / OSCALE
        o[c * QH + loc] = rows
    return o
